# revision 1
# baseline (speedup 1.0000x reference)
"""Trainium2 Bass kernel for CoOccurWithNorm.

Computes per-(image,channel) soft co-occurrence histograms of horizontally
adjacent pixel pairs, normalized by the per-histogram max.

Input  X: [64, 3, 512, 512] fp32, values in [0, 255)
Output:   [64, 3, 256, 256] fp32

Sharding: data-parallel over batch. Core k handles images [8k, 8k+8) ->
24 (image,channel) histograms per core. No cross-core communication.

Algorithm per (b,c):
  hist = sum_c W_c^T @ W_{c+1} over image columns c, accumulated in PSUM,
  where W_c is the [128 rows, 256 bins] soft one-hot (raised-cosine weights
  w0 = (1+cos(pi*f))/2 at bin ix=floor(x), w1 = 1-w0 at ix+1) of column c of
  a 128-row block. Each column's one-hot serves as rhs for chunk c-1 and
  lhsT for chunk c. One-hots are built sparsely by GPSIMD local_scatter
  (2 writes per sample) from precomputed weight/index pair streams.
  Normalization: hist / max(hist) on-device.
"""

import sys
import types
import numpy as np

sys.path.insert(0, "/root/.axon_site/_ro/trn_rl_repo")

import concourse.bass as bass
import concourse.bacc as bacc
import concourse.tile as tile
import concourse.mybir as mybir
import concourse.bass_isa as bass_isa

N_CORES = 8
NBINS = 256
H = 512
W = 512
PB = 128  # partition block (rows per block)
GRP = 7  # columns per local_scatter group (num_elems 7*256=1792, *32 < 2^16)

_PI = float(np.pi)


def install_ntff_hook():
    """Register the axon NTFF profiling hook (missing antenv.axon_hooks shim)."""
    import antenv

    if "antenv.axon_hooks" in sys.modules:
        return
    hooks_mod = types.ModuleType("antenv.axon_hooks")
    _hook = [None]
    hooks_mod.set_axon_ntff_profile_hook = lambda h: _hook.__setitem__(0, h)
    hooks_mod.get_axon_ntff_profile_hook = lambda: _hook[0]
    sys.modules["antenv.axon_hooks"] = hooks_mod
    antenv.axon_hooks = hooks_mod
    try:
        from trn_agent_boot.trn_boot import _ntff_profile_via_ctypes

        hooks_mod.set_axon_ntff_profile_hook(
            _ntff_profile_via_ctypes("/opt/axon/libaxon_pjrt.so")
        )
    except Exception:
        pass


def build_nc(n_bc=24, n_rb=4, debug=False):
    """Build the per-core Bass module.

    n_bc: number of (image,channel) histograms this core computes.
    n_rb: number of 128-row blocks per image (4 for H=512).
    """
    f32 = mybir.dt.float32
    bf16 = mybir.dt.bfloat16
    i16 = mybir.dt.int16

    nc = bacc.Bacc("TRN2", target_bir_lowering=False, debug=debug)

    n_rows = n_bc * n_rb * PB
    XS = nc.dram_tensor("XS", [n_rows, W], f32, kind="ExternalInput")
    OUT = nc.dram_tensor("OUT", [n_bc * NBINS, NBINS], f32, kind="ExternalOutput")

    with tile.TileContext(nc) as tc:
        with (
            tc.tile_pool(name="const", bufs=1) as const_pool,
            tc.tile_pool(name="xin", bufs=2) as xin_pool,
            tc.tile_pool(name="bld", bufs=2) as bld_pool,
            tc.tile_pool(name="wi", bufs=2) as wi_pool,
            tc.tile_pool(name="wt", bufs=3) as wt_pool,
            tc.tile_pool(name="dd", bufs=4) as dd_pool,
            tc.tile_pool(name="wtd", bufs=16) as wtd_pool,
            tc.tile_pool(name="ep", bufs=2) as ep_pool,
            tc.tile_pool(name="psum", bufs=2, space=bass.MemorySpace.PSUM) as psum_pool,
        ):
            # Constant index-offset pattern: for column-pair slot j in [0,1024):
            #   P[j] = 256*((j//2) % GRP) + (j % 2)
            n_grp_full = W // GRP  # full groups of GRP columns
            tail_cols = W - n_grp_full * GRP
            n_grp = n_grp_full + (1 if tail_cols else 0)
            ptile = const_pool.tile([128, n_grp * GRP * 2], i16)
            p4 = ptile[:].rearrange("p (a b t) -> p a b t", b=GRP, t=2)
            nc.gpsimd.iota(
                p4, pattern=[[0, n_grp], [NBINS, GRP], [1, 2]], base=0,
                channel_multiplier=0,
            )
            # bias constant for the Sin activation: cos(pi*f) = -sin(pi*f - pi/2),
            # keeping the Sin argument within the ScalarE table range [-pi, pi]
            sin_bias = const_pool.tile([128, 1], f32)
            nc.vector.memset(sin_bias[:], -_PI / 2.0)
            # dense-builder consts: cos(pi*u) = sin(pi/2 - pi*u) for u in [0,1]
            sin_bias_p = const_pool.tile([128, 1], f32)
            nc.vector.memset(sin_bias_p[:], _PI / 2.0)
            iot = const_pool.tile([128, NBINS], f32)
            nc.gpsimd.iota(iot[:], pattern=[[1, NBINS]], base=0,
                           channel_multiplier=0,
                           allow_small_or_imprecise_dtypes=True)

            with tc.For_i(0, n_bc, 1) as iv:
                epsum = [
                    psum_pool.tile([128, NBINS], f32, tag="eps0", name="eps0"),
                    psum_pool.tile([128, NBINS], f32, tag="eps1", name="eps1"),
                ]
                for rb in range(n_rb):
                    # ---- load one 128-row block of the image ----
                    xt = xin_pool.tile([128, W], f32)
                    nc.sync.dma_start(
                        xt[:], XS[bass.ds(iv * (n_rb * PB) + rb * PB, PB), :]
                    )
                    # ---- build weight + index pair streams ----
                    xc = bld_pool.tile([128, W], f32, tag="xc")
                    nc.vector.tensor_scalar(xc[:], xt[:], 254.999985, None,
                                            op0=mybir.AluOpType.min)
                    # floor/frac without `mod` (not a HW TensorScalar op):
                    # rn = round-to-nearest via the 2^23 magic number, then
                    # correct rn(x) > x cases to get floor exactly.
                    rn = bld_pool.tile([128, W], f32, tag="rn")
                    nc.vector.tensor_scalar(
                        rn[:], xc[:], 8388608.0, 8388608.0,
                        op0=mybir.AluOpType.add, op1=mybir.AluOpType.subtract,
                    )
                    fr0 = bld_pool.tile([128, W], f32, tag="fr0")
                    nc.vector.tensor_sub(fr0[:], xc[:], rn[:])
                    neg = bld_pool.tile([128, W], f32, tag="neg")
                    nc.vector.tensor_scalar(neg[:], fr0[:], 0.0, None,
                                            op0=mybir.AluOpType.is_lt)
                    fr = bld_pool.tile([128, W], f32, tag="fr")
                    nc.vector.tensor_add(fr[:], fr0[:], neg[:])
                    ixf = bld_pool.tile([128, W], f32, tag="ixf")
                    nc.vector.tensor_sub(ixf[:], xc[:], fr[:])
                    cosv = bld_pool.tile([128, W], f32, tag="cosv")
                    nc.scalar.activation(
                        cosv[:], fr[:], mybir.ActivationFunctionType.Sin,
                        bias=sin_bias[:], scale=_PI,
                    )
                    # interleaved (w0, w1) bf16 pairs
                    w01 = wi_pool.tile([128, 2 * W], bf16, tag="w01")
                    nc.vector.tensor_scalar(
                        w01[:, 0 : 2 * W : 2], cosv[:], -0.5, 0.5,
                        op0=mybir.AluOpType.mult, op1=mybir.AluOpType.add,
                    )
                    nc.vector.tensor_scalar(
                        w01[:, 1 : 2 * W : 2], cosv[:], 0.5, 0.5,
                        op0=mybir.AluOpType.mult, op1=mybir.AluOpType.add,
                    )
                    # interleaved (ix, ix) int16 pairs, then += P pattern
                    idx01 = wi_pool.tile([128, 2 * W], i16, tag="idx01")
                    nc.vector.tensor_scalar(
                        idx01[:, 0 : 2 * W : 2], ixf[:], 0.0, None,
                        op0=mybir.AluOpType.add,
                    )
                    nc.vector.tensor_scalar(
                        idx01[:, 1 : 2 * W : 2], ixf[:], 0.0, None,
                        op0=mybir.AluOpType.add,
                    )
                    nc.vector.tensor_tensor(
                        idx01[:], idx01[:], ptile[:, 0 : 2 * W], op=mybir.AluOpType.add
                    )

                    # ---- build one-hots (GPSIMD scatter or DVE dense) ----
                    colap = [None] * W
                    first = rb == 0
                    last = rb == n_rb - 1

                    def chunks_of(g, colap=colap, first=first, last=last):
                        # matmul chunks whose lhsT column lives in group g
                        for cc in range(GRP):
                            c = GRP * g + cc
                            if c >= W - 1:
                                break
                            st = first and c == 0
                            sp = last and c == W - 2
                            for h in range(2):
                                nc.tensor.matmul(
                                    epsum[h][:],
                                    colap[c][:, h * 128 : h * 128 + 128],
                                    colap[c + 1][:],
                                    start=st, stop=sp,
                                )

                    def build_group(g):
                        ncols = min(GRP, W - GRP * g)
                        if g % 5 == 4 and ncols == GRP:
                            # dense build on DVE + ScalarE (offloads GPSIMD):
                            # dc = clip(j - x, -1, 1); s = sin(pi/2*dc);
                            # W = 1 - s^2 = (1+cos(pi*dc))/2, exactly 0 outside support
                            for cc in range(ncols):
                                c = GRP * g + cc
                                dd = dd_pool.tile([128, NBINS], f32, tag="dd")
                                nc.vector.tensor_scalar(
                                    dd[:], iot[:], xc[:, c : c + 1], 1.0,
                                    op0=mybir.AluOpType.subtract,
                                    op1=mybir.AluOpType.min,
                                )
                                nc.vector.tensor_scalar(
                                    dd[:], dd[:], -1.0, None, op0=mybir.AluOpType.max
                                )
                                sv = dd_pool.tile([128, NBINS], f32, tag="sv")
                                nc.scalar.activation(
                                    sv[:], dd[:], mybir.ActivationFunctionType.Sin,
                                    scale=_PI / 2.0,
                                )
                                sq = dd_pool.tile([128, NBINS], f32, tag="sq")
                                nc.vector.tensor_mul(sq[:], sv[:], sv[:])
                                wd = wtd_pool.tile([128, NBINS], bf16, tag="wtd")
                                nc.vector.tensor_scalar(
                                    wd[:], sq[:], -1.0, 1.0,
                                    op0=mybir.AluOpType.mult, op1=mybir.AluOpType.add,
                                )
                                colap[c] = wd[:]
                        else:
                            wt = wt_pool.tile([128, GRP * NBINS], bf16, tag="wt")
                            nc.gpsimd.local_scatter(
                                wt[:],
                                w01[:, 2 * GRP * g : 2 * GRP * g + 2 * ncols],
                                idx01[:, 2 * GRP * g : 2 * GRP * g + 2 * ncols],
                                channels=128,
                                num_elems=GRP * NBINS,
                                num_idxs=2 * ncols,
                            )
                            for cc in range(ncols):
                                colap[GRP * g + cc] = wt[:, cc * NBINS : (cc + 1) * NBINS]

                    for g in range(n_grp):
                        build_group(g)
                        if g > 0:
                            chunks_of(g - 1)
                    chunks_of(n_grp - 1)

                # ---- epilogue: normalize by max and store ----
                mx = ep_pool.tile([128, 2], f32, tag="mx")
                for h in range(2):
                    nc.vector.tensor_reduce(
                        mx[:, h : h + 1], epsum[h][:],
                        axis=mybir.AxisListType.X, op=mybir.AluOpType.max,
                    )
                ar = ep_pool.tile([128, 2], f32, tag="ar")
                nc.gpsimd.partition_all_reduce(
                    ar[:], mx[:], channels=128, reduce_op=bass_isa.ReduceOp.max
                )
                vm128 = ep_pool.tile([128, 1], f32, tag="vm128")
                nc.vector.tensor_reduce(
                    vm128[:], ar[:], axis=mybir.AxisListType.X, op=mybir.AluOpType.max
                )
                rv128 = ep_pool.tile([128, 1], f32, tag="rv128")
                nc.vector.reciprocal(rv128[:], vm128[:])
                outs = ep_pool.tile([128, 2 * NBINS], f32, tag="outs")
                for h in range(2):
                    nc.vector.tensor_scalar(
                        outs[:, h * NBINS : (h + 1) * NBINS], epsum[h][:],
                        rv128[:], None, op0=mybir.AluOpType.mult,
                    )
                    nc.sync.dma_start(
                        OUT[bass.ds(iv * NBINS + h * 128, 128), :],
                        outs[:, h * NBINS : (h + 1) * NBINS],
                    )

    nc.compile()
    return nc


_NC_CACHE = {}


def _get_nc(key=(24, 4)):
    if key not in _NC_CACHE:
        _NC_CACHE[key] = build_nc(n_bc=key[0], n_rb=key[1], debug=False)
    return _NC_CACHE[key]


def kernel(X: np.ndarray) -> np.ndarray:
    """X: [64, 3, 512, 512] fp32 -> [64, 3, 256, 256] fp32."""
    from concourse.bass_utils import run_bass_kernel_spmd

    B, C, Hh, Ww = X.shape
    assert (Hh, Ww) == (H, W)
    per = B // N_CORES  # images per core
    n_bc = per * C

    nc = _get_nc((n_bc, H // PB))

    in_maps = []
    for k in range(N_CORES):
        shard = X[k * per : (k + 1) * per]  # [per, C, H, W]
        in_maps.append(
            {"XS": np.ascontiguousarray(shard.reshape(n_bc * H, W), dtype=np.float32)}
        )

    res = run_bass_kernel_spmd(nc, in_maps, core_ids=list(range(N_CORES)))
    out = np.empty((B, C, NBINS, NBINS), dtype=np.float32)
    for k in range(N_CORES):
        out[k * per : (k + 1) * per] = res.results[k]["OUT"].reshape(
            per, C, NBINS, NBINS
        )
    return out



# revision 3
# speedup vs baseline: 1.8240x; 1.8240x over previous
"""Trainium2 Bass kernel for CoOccurWithNorm (v2: fp8 DoubleRow).

Computes per-(image,channel) soft co-occurrence histograms of horizontally
adjacent pixel pairs, normalized by the per-histogram max.

Input  X: [64, 3, 512, 512] fp32, values in [0, 255)
Output:   [64, 3, 256, 256] fp32

Sharding: data-parallel over batch. Core k handles images [8k, 8k+8) ->
24 (image,channel) histograms per core. No cross-core communication.

Algorithm per (b,c): hist = sum_c W_c^T @ W_{c+1} over image columns c,
accumulated in PSUM, where W_c is the [rows, 256 bins] raised-cosine soft
one-hot of column c. v2 packs one-hots as float8e4 and pairs TWO 128-row
blocks per matmul via MatmulPerfMode.DoubleRow (fp8 k-tile pairing), halving
both PE instruction count and GPSIMD scatter bytes vs the bf16 baseline:

 - one-hot tiles: [128, GRP cols, 2 ktiles, 256 fp8] stored as int16
   (adjacent-bin fp8 pairs packed per uint16); built by GPSIMD local_scatter
   from byte-interleaved fp8 data streams written by DVE (slot A =
   floor(ix/2) always; slot B = (ix+1)/2 for odd ix, else negative = skipped).
 - every dense_mod-th group is instead built densely on DVE (iota-compare)
   to offload the GPSIMD bottleneck.
 - matmuls: lhsT = W_c half [128, 2, 128] fp8, rhs = W_{c+1} [128, 2, 256],
   DoubleRow -> out [128, 256] fp32 accumulating both row blocks at once.
 - epilogue: per-(b,c) max + reciprocal + scale, DMA out.
"""

import sys
import types
import numpy as np

sys.path.insert(0, "/root/.axon_site/_ro/trn_rl_repo")

import concourse.bass as bass
import concourse.bacc as bacc
import concourse.tile as tile
import concourse.mybir as mybir
import concourse.bass_isa as bass_isa

N_CORES = 8
NBINS = 256
H = 512
W = 512
PB = 128
GRP = 7            # columns per scatter group
NG = 74            # groups per block-pair (74*7 = 518 >= 512, cols padded)
WP = NG * GRP      # padded columns (518)
NT = 2             # row blocks (k-tiles) per block-pair
BIG = 8192.0
MAGIC = 8388608.0
_PI = float(np.pi)

f32 = mybir.dt.float32
bf16 = mybir.dt.bfloat16
i16 = mybir.dt.int16
f8 = mybir.dt.float8e4

AOT = mybir.AluOpType


def install_ntff_hook():
    """Register the axon NTFF profiling hook (missing antenv.axon_hooks shim)."""
    import antenv

    if "antenv.axon_hooks" in sys.modules:
        return
    hooks_mod = types.ModuleType("antenv.axon_hooks")
    _hook = [None]
    hooks_mod.set_axon_ntff_profile_hook = lambda h: _hook.__setitem__(0, h)
    hooks_mod.get_axon_ntff_profile_hook = lambda: _hook[0]
    sys.modules["antenv.axon_hooks"] = hooks_mod
    antenv.axon_hooks = hooks_mod
    try:
        from trn_agent_boot.trn_boot import _ntff_profile_via_ctypes

        hooks_mod.set_axon_ntff_profile_hook(
            _ntff_profile_via_ctypes("/opt/axon/libaxon_pjrt.so")
        )
    except Exception:
        pass


def build_nc(n_bc=24, dense_mod=10, debug=False):
    """Build the per-core Bass module.

    n_bc: number of (image,channel) histograms this core computes.
    dense_mod: every dense_mod-th group is built densely on DVE instead of
      GPSIMD local_scatter (0 disables the dense path).
    """
    nc = bacc.Bacc("TRN2", target_bir_lowering=False, debug=debug)

    XS = nc.dram_tensor("XS", [n_bc * H, W], f32, kind="ExternalInput")
    OUT = nc.dram_tensor("OUT", [n_bc * NBINS, NBINS], f32, kind="ExternalOutput")

    n_bp = H // (NT * PB)  # block-pairs per image (2)

    with tile.TileContext(nc) as tc:
        with (
            tc.tile_pool(name="const", bufs=1) as cpool,
            tc.tile_pool(name="xin", bufs=2) as xin_pool,
            tc.tile_pool(name="bld", bufs=2) as bld_pool,
            tc.tile_pool(name="str", bufs=2) as str_pool,
            tc.tile_pool(name="wt", bufs=3) as wt_pool,
            tc.tile_pool(name="dd", bufs=4) as dd_pool,
            tc.tile_pool(name="wtd", bufs=16) as wtd_pool,
            tc.tile_pool(name="ep", bufs=2) as ep_pool,
            tc.tile_pool(name="psum", bufs=2, space=bass.MemorySpace.PSUM) as psum_pool,
        ):
            # ---- constants ----
            sin_bias = cpool.tile([PB, 1], f32)
            nc.vector.memset(sin_bias[:], -_PI / 2.0)
            # natural-layout patterns over (t, g, c): base + c*256 + t*128
            paN0 = cpool.tile([PB, NT * WP], f32)
            nc.gpsimd.iota(paN0[:].rearrange("p (t g c) -> p t g c", t=NT, g=NG),
                           pattern=[[128, NT], [0, NG], [NBINS, GRP]], base=0,
                           channel_multiplier=0,
                           allow_small_or_imprecise_dtypes=True)
            paN1 = cpool.tile([PB, NT * WP], f32)
            nc.gpsimd.iota(paN1[:].rearrange("p (t g c) -> p t g c", t=NT, g=NG),
                           pattern=[[128, NT], [0, NG], [NBINS, GRP]], base=1,
                           channel_multiplier=0,
                           allow_small_or_imprecise_dtypes=True)
            # iota over bins (fp32, 0..255) for dense builds
            iot = cpool.tile([PB, NBINS], f32)
            nc.gpsimd.iota(iot[:], pattern=[[1, NBINS]], base=0,
                           channel_multiplier=0,
                           allow_small_or_imprecise_dtypes=True)

            with tc.For_i(0, n_bc, 1) as iv:
                epsum = [
                    psum_pool.tile([PB, NBINS], f32, tag="eps0", name="eps0"),
                    psum_pool.tile([PB, NBINS], f32, tag="eps1", name="eps1"),
                ]
                for bp in range(n_bp):
                    # ---- load 2 row blocks: xt[p, t, col], pad cols ----
                    xt = xin_pool.tile([PB, NT, WP], f32, tag="xt")
                    nc.sync.dma_start(
                        xt[:, :, 0:W],
                        XS[bass.ds(iv * H + bp * NT * PB, NT * PB), :].rearrange(
                            "(t p) w -> p t w", t=NT),
                    )
                    nc.vector.memset(xt[:, :, W:WP], 128.0)

                    def bld(tg):
                        return bld_pool.tile([PB, NT, WP], f32, tag=tg, name=tg)

                    # ---- prep chain (fp32, natural layout [p, t, col]) ----
                    xc = bld("A")
                    nc.vector.tensor_scalar(xc[:], xt[:], 254.999985, None,
                                            op0=AOT.min)
                    rn = bld("B")
                    nc.vector.tensor_scalar(rn[:], xc[:], MAGIC, MAGIC,
                                            op0=AOT.add, op1=AOT.subtract)
                    fr0 = bld("C")
                    nc.vector.tensor_sub(fr0[:], xc[:], rn[:])
                    ngt = bld("B")
                    nc.vector.tensor_scalar(ngt[:], fr0[:], 0.0, None,
                                            op0=AOT.is_lt)
                    fr = bld("D")
                    nc.vector.tensor_add(fr[:], fr0[:], ngt[:])
                    ixf = bld("ixf")
                    nc.vector.tensor_sub(ixf[:], xc[:], fr[:])
                    cosv = bld("A")
                    nc.scalar.activation(cosv[:], fr[:],
                                         mybir.ActivationFunctionType.Sin,
                                         bias=sin_bias[:], scale=_PI)
                    h = bld("B")
                    nc.vector.tensor_scalar(h[:], ixf[:], 0.5, None, op0=AOT.mult)
                    hn = bld("C")
                    nc.vector.tensor_scalar(hn[:], h[:], MAGIC, MAGIC,
                                            op0=AOT.add, op1=AOT.subtract)
                    gt = bld("D")
                    nc.vector.tensor_tensor(gt[:], hn[:], h[:], op=AOT.is_gt)
                    hf = bld("hf")
                    nc.vector.tensor_sub(hf[:], hn[:], gt[:])
                    ph = bld("C")
                    nc.vector.tensor_sub(ph[:], h[:], hf[:])
                    m0 = bld("m0")  # w0 = 0.5 - 0.5*cosv
                    nc.vector.tensor_scalar(m0[:], cosv[:], -0.5, 0.5,
                                            op0=AOT.mult, op1=AOT.add)
                    m1 = bld("m1")  # w1 = 0.5 + 0.5*cosv
                    nc.vector.tensor_scalar(m1[:], cosv[:], 0.5, 0.5,
                                            op0=AOT.mult, op1=AOT.add)
                    ixp1 = bld("ixp1")  # ix + 1 (dense-path compare scalar)
                    nc.vector.tensor_scalar(ixp1[:], ixf[:], 1.0, None,
                                            op0=AOT.add)

                    # ---- scatter data/idx streams (group-ordered) ----
                    dataAB = str_pool.tile([PB, NG * 28], i16, tag="dataAB")
                    idxAB = str_pool.tile([PB, NG * 28], i16, tag="idxAB")
                    nc.vector.memset(dataAB[:], 0)
                    d8 = dataAB[:].bitcast(f8).rearrange("p (g x) -> p g x", g=NG)

                    def gview(lo, hi, off):
                        vw = d8[:, :, lo + off:hi + off:2]
                        return vw.rearrange("p g (c t) -> p g c t", t=NT)

                    def nat(t_):
                        return t_[:].rearrange("p t (g c) -> p g c t", g=NG)

                    # A even byte: m0*(1-2ph)
                    u = bld("B")
                    nc.vector.tensor_scalar(u[:], ph[:], -2.0, 1.0,
                                            op0=AOT.mult, op1=AOT.add)
                    nc.vector.tensor_mul(gview(0, 28, 0), nat(m0), nat(u))
                    # A odd byte: -2*(ph*cosv) + m1
                    v = bld("B")
                    nc.vector.tensor_mul(v[:], ph[:], cosv[:])
                    nc.vector.scalar_tensor_tensor(gview(0, 28, 1), nat(v), -2.0,
                                                   nat(m1),
                                                   op0=AOT.mult, op1=AOT.add)
                    # B even byte: m1
                    nc.vector.tensor_scalar(gview(28, 56, 0), nat(m1), 0.0, None,
                                            op0=AOT.add)

                    ix_r = idxAB[:].rearrange("p (g x) -> p g x", g=NG)
                    ia = bld("D")  # hf + paN0
                    nc.vector.tensor_add(ia[:], hf[:], paN0[:].rearrange(
                        "p (t w) -> p t w", t=NT))
                    nc.vector.tensor_scalar(
                        ix_r[:, :, 0:14].rearrange("p g (c t) -> p g c t", t=NT),
                        nat(ia), 0.0, None, op0=AOT.add)
                    parT = bld("B")  # 2*ph
                    nc.vector.tensor_scalar(parT[:], ph[:], 2.0, None,
                                            op0=AOT.mult)
                    s1 = bld("D")  # hf + paN1
                    nc.vector.tensor_add(s1[:], hf[:], paN1[:].rearrange(
                        "p (t w) -> p t w", t=NT))
                    s2 = bld("A")  # parT * s1
                    nc.vector.tensor_mul(s2[:], s1[:], parT[:])
                    s3 = bld("D")  # BIG*parT - BIG
                    nc.vector.tensor_scalar(s3[:], parT[:], BIG, -BIG,
                                            op0=AOT.mult, op1=AOT.add)
                    ib = bld("B")
                    nc.vector.tensor_add(ib[:], s2[:], s3[:])
                    nc.vector.tensor_scalar(
                        ix_r[:, :, 14:28].rearrange("p g (c t) -> p g c t", t=NT),
                        nat(ib), 0.0, None, op0=AOT.add)

                    # ---- build one-hots + matmuls ----
                    colap = [None] * WP
                    first = bp == 0
                    last = bp == n_bp - 1

                    def chunks_of(g):
                        for cc in range(GRP):
                            c = GRP * g + cc
                            if c >= W - 1:
                                break
                            st = first and c == 0
                            sp = last and c == W - 2
                            for hh in range(2):
                                nc.tensor.matmul(
                                    epsum[hh][:],
                                    colap[c][:, :, hh * 128:hh * 128 + 128],
                                    colap[c + 1][:, :, :],
                                    start=st, stop=sp,
                                    perf_mode=mybir.MatmulPerfMode.DoubleRow)

                    def build_group(g):
                        if dense_mod and g % dense_mod == dense_mod - 1 and \
                                (g + 1) * GRP <= W:
                            for cc in range(GRP):
                                c = GRP * g + cc
                                wd = wtd_pool.tile([PB, NT, NBINS], f8, tag="wtd")
                                for t in range(NT):
                                    e0 = dd_pool.tile([PB, NBINS], bf16, tag="e0")
                                    nc.vector.tensor_scalar(
                                        e0[:], iot[:], ixf[:, t:t + 1, c:c + 1],
                                        m0[:, t:t + 1, c:c + 1],
                                        op0=AOT.is_equal, op1=AOT.mult)
                                    e1 = dd_pool.tile([PB, NBINS], bf16, tag="e1")
                                    nc.vector.tensor_scalar(
                                        e1[:], iot[:], ixp1[:, t:t + 1, c:c + 1],
                                        m1[:, t:t + 1, c:c + 1],
                                        op0=AOT.is_equal, op1=AOT.mult)
                                    nc.vector.tensor_add(wd[:, t, :], e0[:], e1[:])
                                colap[c] = wd[:]
                        else:
                            wt = wt_pool.tile([PB, GRP * NT * 128], i16, tag="wt")
                            nc.gpsimd.local_scatter(
                                wt[:], dataAB[:, g * 28:(g + 1) * 28],
                                idxAB[:, g * 28:(g + 1) * 28],
                                channels=128, num_elems=GRP * NT * 128,
                                num_idxs=28)
                            wt8 = wt[:].bitcast(f8).rearrange(
                                "p (c t b) -> p c t b", t=NT, b=NBINS)
                            for cc in range(GRP):
                                colap[GRP * g + cc] = wt8[:, cc, :, :]

                    for g in range(NG):
                        build_group(g)
                        if g > 0:
                            chunks_of(g - 1)
                    chunks_of(NG - 1)

                # ---- epilogue: normalize by max and store ----
                mx = ep_pool.tile([PB, 2], f32, tag="mx")
                for hh in range(2):
                    nc.vector.tensor_reduce(
                        mx[:, hh:hh + 1], epsum[hh][:],
                        axis=mybir.AxisListType.X, op=AOT.max)
                ar = ep_pool.tile([PB, 2], f32, tag="ar")
                nc.gpsimd.partition_all_reduce(
                    ar[:], mx[:], channels=128, reduce_op=bass_isa.ReduceOp.max)
                vm128 = ep_pool.tile([PB, 1], f32, tag="vm128")
                nc.vector.tensor_reduce(
                    vm128[:], ar[:], axis=mybir.AxisListType.X, op=AOT.max)
                rv128 = ep_pool.tile([PB, 1], f32, tag="rv128")
                nc.vector.reciprocal(rv128[:], vm128[:])
                outs = ep_pool.tile([PB, 2 * NBINS], f32, tag="outs")
                for hh in range(2):
                    nc.vector.tensor_scalar(
                        outs[:, hh * NBINS:(hh + 1) * NBINS], epsum[hh][:],
                        rv128[:], None, op0=AOT.mult)
                    nc.sync.dma_start(
                        OUT[bass.ds(iv * NBINS + hh * 128, 128), :],
                        outs[:, hh * NBINS:(hh + 1) * NBINS])

    nc.compile()
    return nc


_NC_CACHE = {}


def _get_nc(key=(24, 10)):
    if key not in _NC_CACHE:
        _NC_CACHE[key] = build_nc(n_bc=key[0], dense_mod=key[1], debug=False)
    return _NC_CACHE[key]


def kernel(X: np.ndarray) -> np.ndarray:
    """X: [64, 3, 512, 512] fp32 -> [64, 3, 256, 256] fp32."""
    from concourse.bass_utils import run_bass_kernel_spmd

    B, C, Hh, Ww = X.shape
    assert (Hh, Ww) == (H, W)
    per = B // N_CORES
    n_bc = per * C

    nc = _get_nc((n_bc, 10))

    in_maps = []
    for k in range(N_CORES):
        shard = X[k * per:(k + 1) * per]
        in_maps.append(
            {"XS": np.ascontiguousarray(shard.reshape(n_bc * H, W),
                                        dtype=np.float32)}
        )

    res = run_bass_kernel_spmd(nc, in_maps, core_ids=list(range(N_CORES)))
    out = np.empty((B, C, NBINS, NBINS), dtype=np.float32)
    for k in range(N_CORES):
        out[k * per:(k + 1) * per] = res.results[k]["OUT"].reshape(
            per, C, NBINS, NBINS)
    return out


# revision 4
# speedup vs baseline: 1.8883x; 1.0353x over previous
"""Trainium2 Bass kernel for CoOccurWithNorm (v2: fp8 DoubleRow).

Computes per-(image,channel) soft co-occurrence histograms of horizontally
adjacent pixel pairs, normalized by the per-histogram max.

Input  X: [64, 3, 512, 512] fp32, values in [0, 255)
Output:   [64, 3, 256, 256] fp32

Sharding: data-parallel over batch. Core k handles images [8k, 8k+8) ->
24 (image,channel) histograms per core. No cross-core communication.

Algorithm per (b,c): hist = sum_c W_c^T @ W_{c+1} over image columns c,
accumulated in PSUM, where W_c is the [rows, 256 bins] raised-cosine soft
one-hot of column c. v2 packs one-hots as float8e4 and pairs TWO 128-row
blocks per matmul via MatmulPerfMode.DoubleRow (fp8 k-tile pairing), halving
both PE instruction count and GPSIMD scatter bytes vs the bf16 baseline:

 - one-hot tiles: [128, GRP cols, 2 ktiles, 256 fp8] stored as int16
   (adjacent-bin fp8 pairs packed per uint16); built by GPSIMD local_scatter
   from byte-interleaved fp8 data streams written by DVE (slot A =
   floor(ix/2) always; slot B = (ix+1)/2 for odd ix, else negative = skipped).
 - every dense_mod-th group is instead built densely on DVE (iota-compare)
   to offload the GPSIMD bottleneck.
 - matmuls: lhsT = W_c half [128, 2, 128] fp8, rhs = W_{c+1} [128, 2, 256],
   DoubleRow -> out [128, 256] fp32 accumulating both row blocks at once.
 - epilogue: per-(b,c) max + reciprocal + scale, DMA out.
"""

import sys
import types
import numpy as np

sys.path.insert(0, "/root/.axon_site/_ro/trn_rl_repo")

import concourse.bass as bass
import concourse.bacc as bacc
import concourse.tile as tile
import concourse.mybir as mybir
import concourse.bass_isa as bass_isa

N_CORES = 8
NBINS = 256
H = 512
W = 512
PB = 128
GRP = 7            # columns per scatter group
NG = 74            # groups per block-pair (74*7 = 518 >= 512, cols padded)
WP = NG * GRP      # padded columns (518)
NT = 2             # row blocks (k-tiles) per block-pair
BIG = 8192.0
MAGIC = 8388608.0
_PI = float(np.pi)

f32 = mybir.dt.float32
bf16 = mybir.dt.bfloat16
i16 = mybir.dt.int16
f8 = mybir.dt.float8e4

AOT = mybir.AluOpType


def install_ntff_hook():
    """Register the axon NTFF profiling hook (missing antenv.axon_hooks shim)."""
    import antenv

    if "antenv.axon_hooks" in sys.modules:
        return
    hooks_mod = types.ModuleType("antenv.axon_hooks")
    _hook = [None]
    hooks_mod.set_axon_ntff_profile_hook = lambda h: _hook.__setitem__(0, h)
    hooks_mod.get_axon_ntff_profile_hook = lambda: _hook[0]
    sys.modules["antenv.axon_hooks"] = hooks_mod
    antenv.axon_hooks = hooks_mod
    try:
        from trn_agent_boot.trn_boot import _ntff_profile_via_ctypes

        hooks_mod.set_axon_ntff_profile_hook(
            _ntff_profile_via_ctypes("/opt/axon/libaxon_pjrt.so")
        )
    except Exception:
        pass


def build_nc(n_bc=24, dense_mod=14, debug=False):
    """Build the per-core Bass module.

    n_bc: number of (image,channel) histograms this core computes.
    dense_mod: every dense_mod-th group is built densely on DVE instead of
      GPSIMD local_scatter (0 disables the dense path).
    """
    nc = bacc.Bacc("TRN2", target_bir_lowering=False, debug=debug)

    XS = nc.dram_tensor("XS", [n_bc * H, W], f32, kind="ExternalInput")
    OUT = nc.dram_tensor("OUT", [n_bc * NBINS, NBINS], f32, kind="ExternalOutput")

    n_bp = H // (NT * PB)  # block-pairs per image (2)

    with tile.TileContext(nc) as tc:
        with (
            tc.tile_pool(name="const", bufs=1) as cpool,
            tc.tile_pool(name="xin", bufs=2) as xin_pool,
            tc.tile_pool(name="bld", bufs=2) as bld_pool,
            tc.tile_pool(name="str", bufs=2) as str_pool,
            tc.tile_pool(name="wt", bufs=3) as wt_pool,
            tc.tile_pool(name="dd", bufs=4) as dd_pool,
            tc.tile_pool(name="wtd", bufs=16) as wtd_pool,
            tc.tile_pool(name="ep", bufs=2) as ep_pool,
            tc.tile_pool(name="psum", bufs=2, space=bass.MemorySpace.PSUM) as psum_pool,
        ):
            # ---- constants ----
            sin_bias = cpool.tile([PB, 1], f32)
            nc.vector.memset(sin_bias[:], -_PI / 2.0)
            # natural-layout patterns over (t, g, c): base + c*256 + t*128
            paN0 = cpool.tile([PB, NT * WP], f32)
            nc.gpsimd.iota(paN0[:].rearrange("p (t g c) -> p t g c", t=NT, g=NG),
                           pattern=[[128, NT], [0, NG], [NBINS, GRP]], base=0,
                           channel_multiplier=0,
                           allow_small_or_imprecise_dtypes=True)
            paN1 = cpool.tile([PB, NT * WP], f32)
            nc.gpsimd.iota(paN1[:].rearrange("p (t g c) -> p t g c", t=NT, g=NG),
                           pattern=[[128, NT], [0, NG], [NBINS, GRP]], base=1,
                           channel_multiplier=0,
                           allow_small_or_imprecise_dtypes=True)
            # iota over bins (fp32, 0..255) for dense builds
            iot = cpool.tile([PB, NBINS], f32)
            nc.gpsimd.iota(iot[:], pattern=[[1, NBINS]], base=0,
                           channel_multiplier=0,
                           allow_small_or_imprecise_dtypes=True)

            with tc.For_i(0, n_bc, 1) as iv:
                epsum = [
                    psum_pool.tile([PB, NBINS], f32, tag="eps0", name="eps0"),
                    psum_pool.tile([PB, NBINS], f32, tag="eps1", name="eps1"),
                ]
                for bp in range(n_bp):
                    # ---- load 2 row blocks: xt[p, t, col], pad cols ----
                    xt = xin_pool.tile([PB, NT, WP], f32, tag="xt")
                    nc.sync.dma_start(
                        xt[:, :, 0:W],
                        XS[bass.ds(iv * H + bp * NT * PB, NT * PB), :].rearrange(
                            "(t p) w -> p t w", t=NT),
                    )
                    nc.vector.memset(xt[:, :, W:WP], 128.0)

                    def bld(tg):
                        return bld_pool.tile([PB, NT, WP], f32, tag=tg, name=tg)

                    # ---- prep chain (fp32, natural layout [p, t, col]) ----
                    xc = bld("A")
                    nc.vector.tensor_scalar(xc[:], xt[:], 254.999985, None,
                                            op0=AOT.min)
                    rn = bld("B")
                    nc.vector.tensor_scalar(rn[:], xc[:], MAGIC, MAGIC,
                                            op0=AOT.add, op1=AOT.subtract)
                    fr0 = bld("C")
                    nc.vector.tensor_sub(fr0[:], xc[:], rn[:])
                    ngt = bld("B")
                    nc.vector.tensor_scalar(ngt[:], fr0[:], 0.0, None,
                                            op0=AOT.is_lt)
                    fr = bld("D")
                    nc.vector.tensor_add(fr[:], fr0[:], ngt[:])
                    ixf = bld("ixf")
                    nc.vector.tensor_sub(ixf[:], xc[:], fr[:])
                    cosv = bld("A")
                    nc.scalar.activation(cosv[:], fr[:],
                                         mybir.ActivationFunctionType.Sin,
                                         bias=sin_bias[:], scale=_PI)
                    h = bld("B")
                    nc.vector.tensor_scalar(h[:], ixf[:], 0.5, None, op0=AOT.mult)
                    hn = bld("C")
                    nc.vector.tensor_scalar(hn[:], h[:], MAGIC, MAGIC,
                                            op0=AOT.add, op1=AOT.subtract)
                    gt = bld("D")
                    nc.vector.tensor_tensor(gt[:], hn[:], h[:], op=AOT.is_gt)
                    hf = bld("hf")
                    nc.vector.tensor_sub(hf[:], hn[:], gt[:])
                    ph = bld("C")
                    nc.vector.tensor_sub(ph[:], h[:], hf[:])
                    m0 = bld("m0")  # w0 = 0.5 - 0.5*cosv
                    nc.vector.tensor_scalar(m0[:], cosv[:], -0.5, 0.5,
                                            op0=AOT.mult, op1=AOT.add)
                    m1 = bld("m1")  # w1 = 0.5 + 0.5*cosv
                    nc.vector.tensor_scalar(m1[:], cosv[:], 0.5, 0.5,
                                            op0=AOT.mult, op1=AOT.add)
                    ixp1 = bld("ixp1")  # ix + 1 (dense-path compare scalar)
                    nc.vector.tensor_scalar(ixp1[:], ixf[:], 1.0, None,
                                            op0=AOT.add)

                    # ---- scatter data/idx streams (group-ordered) ----
                    dataAB = str_pool.tile([PB, NG * 28], i16, tag="dataAB")
                    idxAB = str_pool.tile([PB, NG * 28], i16, tag="idxAB")
                    nc.vector.memset(dataAB[:], 0)
                    d8 = dataAB[:].bitcast(f8).rearrange("p (g x) -> p g x", g=NG)

                    def gview(lo, hi, off):
                        vw = d8[:, :, lo + off:hi + off:2]
                        return vw.rearrange("p g (c t) -> p g c t", t=NT)

                    def nat(t_):
                        return t_[:].rearrange("p t (g c) -> p g c t", g=NG)

                    # A even byte: m0*(1-2ph)
                    u = bld("B")
                    nc.vector.tensor_scalar(u[:], ph[:], -2.0, 1.0,
                                            op0=AOT.mult, op1=AOT.add)
                    nc.vector.tensor_mul(gview(0, 28, 0), nat(m0), nat(u))
                    # A odd byte: -2*(ph*cosv) + m1
                    v = bld("B")
                    nc.vector.tensor_mul(v[:], ph[:], cosv[:])
                    nc.vector.scalar_tensor_tensor(gview(0, 28, 1), nat(v), -2.0,
                                                   nat(m1),
                                                   op0=AOT.mult, op1=AOT.add)
                    # B even byte: m1
                    nc.vector.tensor_scalar(gview(28, 56, 0), nat(m1), 0.0, None,
                                            op0=AOT.add)

                    ix_r = idxAB[:].rearrange("p (g x) -> p g x", g=NG)
                    ia = bld("D")  # hf + paN0
                    nc.vector.tensor_add(ia[:], hf[:], paN0[:].rearrange(
                        "p (t w) -> p t w", t=NT))
                    nc.vector.tensor_scalar(
                        ix_r[:, :, 0:14].rearrange("p g (c t) -> p g c t", t=NT),
                        nat(ia), 0.0, None, op0=AOT.add)
                    parT = bld("B")  # 2*ph
                    nc.vector.tensor_scalar(parT[:], ph[:], 2.0, None,
                                            op0=AOT.mult)
                    s1 = bld("D")  # hf + paN1
                    nc.vector.tensor_add(s1[:], hf[:], paN1[:].rearrange(
                        "p (t w) -> p t w", t=NT))
                    s2 = bld("A")  # parT * s1
                    nc.vector.tensor_mul(s2[:], s1[:], parT[:])
                    s3 = bld("D")  # BIG*parT - BIG
                    nc.vector.tensor_scalar(s3[:], parT[:], BIG, -BIG,
                                            op0=AOT.mult, op1=AOT.add)
                    ib = bld("B")
                    nc.vector.tensor_add(ib[:], s2[:], s3[:])
                    nc.vector.tensor_scalar(
                        ix_r[:, :, 14:28].rearrange("p g (c t) -> p g c t", t=NT),
                        nat(ib), 0.0, None, op0=AOT.add)

                    # ---- build one-hots + matmuls ----
                    colap = [None] * WP
                    first = bp == 0
                    last = bp == n_bp - 1

                    def chunks_of(g):
                        for cc in range(GRP):
                            c = GRP * g + cc
                            if c >= W - 1:
                                break
                            st = first and c == 0
                            sp = last and c == W - 2
                            for hh in range(2):
                                nc.tensor.matmul(
                                    epsum[hh][:],
                                    colap[c][:, :, hh * 128:hh * 128 + 128],
                                    colap[c + 1][:, :, :],
                                    start=st, stop=sp,
                                    perf_mode=mybir.MatmulPerfMode.DoubleRow)

                    def build_group(g):
                        if dense_mod and g % dense_mod == dense_mod - 1 and \
                                (g + 1) * GRP <= W:
                            for cc in range(GRP):
                                c = GRP * g + cc
                                wd = wtd_pool.tile([PB, NT, NBINS], f8, tag="wtd")
                                for t in range(NT):
                                    e0 = dd_pool.tile([PB, NBINS], bf16, tag="e0")
                                    nc.vector.tensor_scalar(
                                        e0[:], iot[:], ixf[:, t:t + 1, c:c + 1],
                                        m0[:, t:t + 1, c:c + 1],
                                        op0=AOT.is_equal, op1=AOT.mult)
                                    e1 = dd_pool.tile([PB, NBINS], bf16, tag="e1")
                                    nc.vector.tensor_scalar(
                                        e1[:], iot[:], ixp1[:, t:t + 1, c:c + 1],
                                        m1[:, t:t + 1, c:c + 1],
                                        op0=AOT.is_equal, op1=AOT.mult)
                                    nc.vector.tensor_add(wd[:, t, :], e0[:], e1[:])
                                colap[c] = wd[:]
                        else:
                            wt = wt_pool.tile([PB, GRP * NT * 128], i16, tag="wt")
                            nc.gpsimd.local_scatter(
                                wt[:], dataAB[:, g * 28:(g + 1) * 28],
                                idxAB[:, g * 28:(g + 1) * 28],
                                channels=128, num_elems=GRP * NT * 128,
                                num_idxs=28)
                            wt8 = wt[:].bitcast(f8).rearrange(
                                "p (c t b) -> p c t b", t=NT, b=NBINS)
                            for cc in range(GRP):
                                colap[GRP * g + cc] = wt8[:, cc, :, :]

                    for g in range(NG):
                        build_group(g)
                        if g > 0:
                            chunks_of(g - 1)
                    chunks_of(NG - 1)

                # ---- epilogue: normalize by max and store ----
                mx = ep_pool.tile([PB, 2], f32, tag="mx")
                for hh in range(2):
                    nc.vector.tensor_reduce(
                        mx[:, hh:hh + 1], epsum[hh][:],
                        axis=mybir.AxisListType.X, op=AOT.max)
                ar = ep_pool.tile([PB, 2], f32, tag="ar")
                nc.gpsimd.partition_all_reduce(
                    ar[:], mx[:], channels=128, reduce_op=bass_isa.ReduceOp.max)
                vm128 = ep_pool.tile([PB, 1], f32, tag="vm128")
                nc.vector.tensor_reduce(
                    vm128[:], ar[:], axis=mybir.AxisListType.X, op=AOT.max)
                rv128 = ep_pool.tile([PB, 1], f32, tag="rv128")
                nc.vector.reciprocal(rv128[:], vm128[:])
                outs = ep_pool.tile([PB, 2 * NBINS], f32, tag="outs")
                for hh in range(2):
                    nc.vector.tensor_scalar(
                        outs[:, hh * NBINS:(hh + 1) * NBINS], epsum[hh][:],
                        rv128[:], None, op0=AOT.mult)
                    nc.sync.dma_start(
                        OUT[bass.ds(iv * NBINS + hh * 128, 128), :],
                        outs[:, hh * NBINS:(hh + 1) * NBINS])

    nc.compile()
    return nc


_NC_CACHE = {}


def _get_nc(key=(24, 14)):
    if key not in _NC_CACHE:
        _NC_CACHE[key] = build_nc(n_bc=key[0], dense_mod=key[1], debug=False)
    return _NC_CACHE[key]


def kernel(X: np.ndarray) -> np.ndarray:
    """X: [64, 3, 512, 512] fp32 -> [64, 3, 256, 256] fp32."""
    from concourse.bass_utils import run_bass_kernel_spmd

    B, C, Hh, Ww = X.shape
    assert (Hh, Ww) == (H, W)
    per = B // N_CORES
    n_bc = per * C

    nc = _get_nc((n_bc, 14))

    in_maps = []
    for k in range(N_CORES):
        shard = X[k * per:(k + 1) * per]
        in_maps.append(
            {"XS": np.ascontiguousarray(shard.reshape(n_bc * H, W),
                                        dtype=np.float32)}
        )

    res = run_bass_kernel_spmd(nc, in_maps, core_ids=list(range(N_CORES)))
    out = np.empty((B, C, NBINS, NBINS), dtype=np.float32)
    for k in range(N_CORES):
        out[k * per:(k + 1) * per] = res.results[k]["OUT"].reshape(
            per, C, NBINS, NBINS)
    return out
